# revision 15
# baseline (speedup 1.0000x reference)
"""NoisyHadamardLinear TRN2 kernel — bf16 main + fp8 DoubleRow hybrid.

Same structure as kernel.py (Hadamard folded into W on host, pre-transposed
bf16 operands, single streaming matmul), plus: the first 2 of 32 contraction
d-blocks (d 0..255) are computed in fp8e4 with MatmulPerfMode.DoubleRow
(2 fp8 rows per PE cycle -> 0.5 cyc/row).  Scale bookkeeping: ALL W (bf16
and fp8) and the bias are pre-scaled x1024 on the host (free for bf16; gives
fp8 its mantissa window), x is unscaled, the host divides y by 1024 (exact).

DoubleRow mechanics: lhsT free [2, 64] = [A | B], rhs free [2, f] = [Wa|Wb],
out[64, f] = A^T Wa + B^T Wb (contraction depth 256 per instruction), and
the ISA only allows DR destinations on psum partitions 0..63.  So per
128-token tile: token-half 0 accumulates directly into the main psum bank
(partitions 0..63); token-half 1 goes to a scratch [64, x] psum at base 0,
is evicted by ACT to SBUF, partition-shifted 0..63 -> 64..127 by an
SBUF->SBUF DMA, and merged by a second DVE add at eviction time — all off
the PE critical path.
"""
import numpy as np
import ml_dtypes

import concourse.bacc as bacc
import concourse.mybir as mybir
import concourse.tile as tile
from concourse.bass_utils import run_bass_kernel_spmd

P = 128
OS = 512
bf16 = mybir.dt.bfloat16
fp8 = mybir.dt.float8e4
f32 = mybir.dt.float32

N_CORES = 8
B, S, D, O = 2, 4096, 4096, 4096
T_PER_CORE = (B * S) // N_CORES
HAD_BLOCK = 1024
NF8 = 4                    # fp8 d-blocks (d 0..511), must be even (DR pairs)
SW = 1024.0                # host-side W/bias scale (power of 2, exact)


def build_kernel(T=T_PER_CORE, D=D, O=O, num_devices=N_CORES):
    NDC = D // P - NF8                     # 30 clean (bf16) d-tiles
    NT = T // P                            # 8 t-tiles
    NOS = O // OS                          # 8 o-slabs
    DR = mybir.MatmulPerfMode.DoubleRow

    nc = bacc.Bacc("TRN2", target_bir_lowering=False, debug=False,
                   num_devices=num_devices, dynamic_dma_scratch_size=2048)
    xT = nc.dram_tensor("xT", [NDC * P, T], bf16, kind="ExternalInput")
    Wp = nc.dram_tensor("Wp", [P, NDC, O], bf16, kind="ExternalInput")
    x8d = nc.dram_tensor("x8", [P, NT * 2 * NF8 * 64], fp8,
                         kind="ExternalInput")
    W8d = nc.dram_tensor("W8", [P, NOS * 4 * NF8 * P], fp8,
                         kind="ExternalInput")
    brep = nc.dram_tensor("brep", [P, O], f32, kind="ExternalInput")
    y = nc.dram_tensor("y", [T, O], f32, kind="ExternalOutput")

    with tile.TileContext(nc) as tc:
        with tc.tile_pool(name="xp", bufs=NDC) as xp, \
             tc.tile_pool(name="w0p", bufs=NDC) as w0p, \
             tc.tile_pool(name="wsp", bufs=2) as wsp, \
             tc.tile_pool(name="f8p", bufs=1) as f8p, \
             tc.tile_pool(name="bp", bufs=1) as bp, \
             tc.tile_pool(name="dp", bufs=1) as dp, \
             tc.tile_pool(name="s8p", bufs=2) as s8p, \
             tc.tile_pool(name="s8hp", bufs=2) as s8hp, \
             tc.tile_pool(name="yop", bufs=4) as yop, \
             tc.tile_pool(name="psp", bufs=7, space="PSUM") as psp, \
             tc.tile_pool(name="ps8p", bufs=1, space="PSUM") as ps8p:
            # warm-up spin: p-state ramp completes while first DMAs fly
            dummy = dp.tile([P, P], bf16)
            nc.gpsimd.memset(dummy[:], 0.0)
            wps = psp.tile([P, P], f32, tag="ps", name="warm")
            for _ in range(29):
                nc.tensor.matmul(wps[:], dummy[:], dummy[:],
                                 start=True, stop=True)

            xt = [xp.tile([P, T], bf16, tag="x", name=f"x{dt}")
                  for dt in range(NDC)]
            w0 = [w0p.tile([P, OS], bf16, tag="w0", name=f"w0_{dt}")
                  for dt in range(NDC)]
            for dt in range(NDC):
                nc.sync.dma_start(xt[dt][:], xT.ap()[dt * P:(dt + 1) * P, :])
                nc.sync.dma_start(w0[dt][:], Wp.ap()[:, dt:dt + 1, 0:OS])
            # fp8 operands: x8 whole + W8's o-slab-0 slice before ws1 so
            # o-slab 0's DoubleRow matmuls aren't gated on the big slabs
            x8t = f8p.tile([P, NT * 2 * NF8 * 64], fp8, tag="x8")
            W8t = f8p.tile([P, NOS * 4 * NF8 * P], fp8, tag="W8")
            W8OS = 4 * NF8 * P             # 1024 cols per o-slab
            nc.sync.dma_start(x8t[:], x8d.ap())
            nc.sync.dma_start(W8t[:, 0:W8OS], W8d.ap()[:, 0:W8OS])

            brt = bp.tile([P, O], f32)

            def load_bias(os_):
                nc.sync.dma_start(brt[:, os_ * OS:(os_ + 1) * OS],
                                  brep.ap()[:, os_ * OS:(os_ + 1) * OS])

            load_bias(0)
            ws = {}
            for os_ in (1, 2):
                ws[os_] = wsp.tile([P, NDC * OS], bf16, tag="ws",
                                   name=f"ws{os_}")
                nc.sync.dma_start(
                    ws[os_][:], Wp.ap()[:, :, os_ * OS:(os_ + 1) * OS])
                if os_ == 1:
                    nc.sync.dma_start(W8t[:, W8OS:], W8d.ap()[:, W8OS:])
                load_bias(os_)

            NPAIR = NF8 // 2

            def dr_mm(out_ap, tt, th, os_, oq, pair, start, stop):
                c0 = ((tt * 2 + th) * NPAIR + pair) * P
                lhsT = x8t[:, c0:c0 + P].rearrange(
                    "p (two m) -> p two m", two=2)
                r0 = ((os_ * 4 + oq) * NPAIR + pair) * 256
                rhs = W8t[:, r0:r0 + 256].rearrange(
                    "p (two f) -> p two f", two=2)
                nc.tensor.matmul(out_ap, lhsT, rhs, start=start, stop=stop,
                                 perf_mode=DR, skip_group_check=True)

            def th1_partial(tt, os_):
                """token-half-1 fp8 partial: DR (base 0) -> ACT -> SBUF
                partition-shift DMA to 64..127.  Returns s8h tile."""
                ps8 = ps8p.tile([64, OS], f32, tag="ps8",
                                name=f"ps8_{os_}_{tt}")
                for oq in range(4):
                    for pair in range(NPAIR):
                        dr_mm(ps8[:, oq * P:(oq + 1) * P], tt, 1, os_, oq,
                              pair, start=(pair == 0),
                              stop=(pair == NPAIR - 1))
                s8 = s8p.tile([64, OS], f32, tag="s8")
                nc.scalar.copy(s8[:], ps8[:])
                s8h = s8hp.tile([P, OS], f32, tag="s8h")
                nc.sync.dma_start(s8h[64:P, :], s8[:])
                return s8h

            def th0_into(py_t, tt, os_, off=0, cw=OS):
                oqs = list(range(off // P, (off + cw) // P))
                for i, oq in enumerate(oqs):
                    for pair in range(NPAIR):
                        dr_mm(py_t[0:64, oq * P - off:(oq + 1) * P - off],
                              tt, 0, os_, oq, pair, start=False,
                              stop=(i == len(oqs) - 1 and
                                    pair == NPAIR - 1))

            def evict(py_t, tt, os_, s8h):
                yo = yop.tile([P, OS], f32, tag="yo")
                nc.vector.tensor_add(yo[:], py_t[:],
                                     brt[:, os_ * OS:(os_ + 1) * OS])
                nc.vector.tensor_add(yo[64:P, :], yo[64:P, :], s8h[64:P, :])
                nc.sync.dma_start(
                    y.ap()[tt * P:(tt + 1) * P, os_ * OS:(os_ + 1) * OS],
                    yo[:])

            def bf16_chain(py_t, tt, w_ap, off=0, cw=OS):
                for dt in range(NDC):
                    nc.tensor.matmul(py_t[:], xt[dt][:, tt * P:(tt + 1) * P],
                                     w_ap[:, dt * OS + off:dt * OS + off + cw],
                                     start=(dt == 0), stop=False)

            # o-slab 0: d-major over 7 t-tiles (psum pool is 7), so each
            # DMA round (x d-tile + W chunk) feeds 7 matmuls and the PE
            # never starves during the streaming phase
            py = [psp.tile([P, OS], f32, tag="ps", name=f"ps0_{tt}")
                  for tt in range(7)]
            for dt in range(NDC):
                for tt in range(7):
                    nc.tensor.matmul(py[tt][:],
                                     xt[dt][:, tt * P:(tt + 1) * P],
                                     w0[dt][:],
                                     start=(dt == 0), stop=False)

            def eb(tt):
                """finish + evict an o-slab-0 d-major tile"""
                s8h = th1_partial(tt, 0)
                th0_into(py[tt], tt, 0)
                evict(py[tt], tt, 0, s8h)

            def job(tt, os_, th1_late=False):
                """one full (tt, os) tile; th1 partial normally first (its
                ACT + shift DMA complete during the bf16 chain); th1_late
                puts it after the chain to space consecutive th1s apart in
                the o-slab-0 eviction zone (single ps8 buffer)"""
                s8h = None if th1_late else th1_partial(tt, os_)
                py_t = psp.tile([P, OS], f32, tag="ps",
                                name=f"ps{os_}_{tt}")
                if os_ == 0:
                    for dt in range(NDC):
                        nc.tensor.matmul(py_t[:],
                                         xt[dt][:, tt * P:(tt + 1) * P],
                                         w0[dt][:],
                                         start=(dt == 0), stop=False)
                else:
                    bf16_chain(py_t, tt, ws[os_])
                if th1_late:
                    s8h = th1_partial(tt, os_)
                th0_into(py_t, tt, os_)
                evict(py_t, tt, os_, s8h)

            def tail_job(tt, os_):
                s8h = th1_partial(tt, os_)
                # pre-combine bias + fp8 partial during the chains so each
                # chunk eviction is a single DVE add on the critical tail
                xc = s8hp.tile([P, OS], f32, tag="s8h", name="xcomb")
                b0 = os_ * OS
                nc.vector.tensor_copy(xc[0:64, :], brt[0:64, b0:b0 + OS])
                nc.vector.tensor_add(xc[64:P, :], brt[64:P, b0:b0 + OS],
                                     s8h[64:P, :])
                off = 0
                for h, cw in enumerate((256, 128, 128)):
                    ph = psp.tile([P, cw], f32, tag="ps",
                                  name=f"ps_tail{h}")
                    bf16_chain(ph, tt, ws[os_], off, cw)
                    th0_into(ph, tt, os_, off, cw)
                    yo = yop.tile([P, cw], f32, tag="yo2")
                    nc.vector.tensor_add(yo[:], ph[:], xc[:, off:off + cw])
                    nc.sync.dma_start(
                        y.ap()[tt * P:(tt + 1) * P, b0 + off:b0 + off + cw],
                        yo[:])
                    off += cw

            # interleave o-slab-0 evictions with the following full chains:
            # each eviction block is followed by a ~6.4us chain, which hides
            # the single-buffered fp8-scratch (ps8/s8/s8h) turnaround
            eb(0)
            job(7, 0)
            stagger = [(0, 1), (1, 1), (2, 1), (3, 1), (4, 1), (5, 1)]
            for k, (jt, jo) in enumerate(stagger):
                eb(k + 1)
                job(jt, jo)
            rest = [(6, 1), (7, 1)] + [(tt, os_) for os_ in range(2, NOS)
                                       for tt in range(NT)]
            for (jt, jo) in rest:
                if jo == NOS - 1 and jt == NT - 1:
                    tail_job(jt, jo)
                else:
                    job(jt, jo)
                if jt == NT - 1 and jo + 2 < NOS:
                    ws[jo + 2] = wsp.tile([P, NDC * OS], bf16, tag="ws",
                                          name=f"ws{jo + 2}")
                    nc.sync.dma_start(
                        ws[jo + 2][:],
                        Wp.ap()[:, :, (jo + 2) * OS:(jo + 3) * OS])
                    load_bias(jo + 2)
    nc.compile()
    return nc


_CACHED_NC = None


def _get_nc():
    global _CACHED_NC
    if _CACHED_NC is None:
        _CACHED_NC = build_kernel()
    return _CACHED_NC


def _fwht_rows(a, block):
    shape = a.shape
    a = a.reshape(-1, block).copy()
    h = 1
    while h < block:
        a = a.reshape(-1, block // (2 * h), 2, h)
        s = a[:, :, 0, :] + a[:, :, 1, :]
        d = a[:, :, 0, :] - a[:, :, 1, :]
        a = np.stack([s, d], axis=2)
        h *= 2
    return a.reshape(shape)


def kernel(x, W, b):
    x = np.asarray(x, dtype=np.float32)
    W = np.asarray(W, dtype=np.float32)
    b = np.asarray(b, dtype=np.float32)
    assert x.shape == (B, S, D) and W.shape == (O, D) and b.shape == (O,)

    nc = _get_nc()
    NDC = D // P - NF8
    e4m3 = ml_dtypes.float8_e4m3
    DS = NF8 * P                           # fp8 d-range (256)

    # W' = FWHT_1024(W rows)/32, then x1024 (exact host rescale at the end)
    Wf = _fwht_rows(W, HAD_BLOCK) * np.float32(SW / 32.0)
    WT = Wf.T                              # [d, o]
    Wpk = np.ascontiguousarray(
        WT[DS:].astype(ml_dtypes.bfloat16)
        .reshape(NDC, P, O).transpose(1, 0, 2))
    # fp8 blocks: [k][os][oq][pair][half][128]
    NPAIR = NF8 // 2
    W8v = WT[:DS].astype(e4m3).reshape(NPAIR, 2, P, O // OS, 4, P)
    W8pk = np.ascontiguousarray(
        W8v.transpose(2, 3, 4, 0, 1, 5)).reshape(P, (O // OS) * 4 * NF8 * P)
    brep = np.ascontiguousarray(
        np.broadcast_to((b * np.float32(SW)).reshape(1, O), (P, O)),
        dtype=np.float32)

    xf = x.reshape(B * S, D)
    in_maps = []
    for c in range(N_CORES):
        xc = xf[c * T_PER_CORE:(c + 1) * T_PER_CORE]
        xTc = np.ascontiguousarray(xc[:, DS:].astype(ml_dtypes.bfloat16).T)
        # fp8 part: [k][tt][th][pair][half][64]
        x8v = (xc[:, :DS].astype(e4m3).T
               .reshape(NPAIR, 2, P, T_PER_CORE // P, 2, 64))
        x8pk = np.ascontiguousarray(
            x8v.transpose(2, 3, 4, 0, 1, 5)).reshape(
                P, (T_PER_CORE // P) * 2 * NF8 * 64)
        in_maps.append({
            "xT": xTc, "Wp": Wpk, "x8": x8pk, "W8": W8pk, "brep": brep,
        })
    res = run_bass_kernel_spmd(nc, in_maps, core_ids=list(range(N_CORES)))
    yv = np.concatenate([res.results[c]["y"] for c in range(N_CORES)], axis=0)
    return (yv.reshape(B, S, O) * np.float32(1.0 / SW)).astype(
        np.float32, copy=False)


# revision 17
# speedup vs baseline: 1.0040x; 1.0040x over previous
"""NoisyHadamardLinear TRN2 kernel — bf16 main + fp8 DoubleRow hybrid.

Hadamard folded into W on host (y = x @ (W H_bd)^T + b), operands shipped
pre-transposed/pre-packed, one streaming matmul per core (data-parallel over
8192 tokens), plus: the first 4 of 32 contraction
d-blocks (d 0..511) are computed in fp8e4 with MatmulPerfMode.DoubleRow
(2 fp8 rows per PE cycle -> 0.5 cyc/row).  Scale bookkeeping: ALL W (bf16
and fp8) and the bias are pre-scaled x1024 on the host (free for bf16; gives
fp8 its mantissa window), x is unscaled, the host divides y by 1024 (exact).

DoubleRow mechanics: lhsT free [2, 64] = [A | B], rhs free [2, f] = [Wa|Wb],
out[64, f] = A^T Wa + B^T Wb (contraction depth 256 per instruction), and
the ISA only allows DR destinations on psum partitions 0..63.  So per
128-token tile: token-half 0 accumulates directly into the main psum bank
(partitions 0..63); token-half 1 goes to a scratch [64, x] psum at base 0,
is evicted by ACT to SBUF, partition-shifted 0..63 -> 64..127 by an
SBUF->SBUF DMA, and merged by a second DVE add at eviction time — all off
the PE critical path.
"""
import numpy as np
import ml_dtypes

import concourse.bacc as bacc
import concourse.mybir as mybir
import concourse.tile as tile
from concourse.bass_utils import run_bass_kernel_spmd

P = 128
OS = 512
bf16 = mybir.dt.bfloat16
fp8 = mybir.dt.float8e4
f32 = mybir.dt.float32

N_CORES = 8
B, S, D, O = 2, 4096, 4096, 4096
T_PER_CORE = (B * S) // N_CORES
HAD_BLOCK = 1024
NF8 = 4                    # fp8 d-blocks (d 0..511), must be even (DR pairs)
SW = 1024.0                # host-side W/bias scale (power of 2, exact)


def build_kernel(T=T_PER_CORE, D=D, O=O, num_devices=N_CORES):
    NDC = D // P - NF8                     # 30 clean (bf16) d-tiles
    NT = T // P                            # 8 t-tiles
    NOS = O // OS                          # 8 o-slabs
    DR = mybir.MatmulPerfMode.DoubleRow

    nc = bacc.Bacc("TRN2", target_bir_lowering=False, debug=False,
                   num_devices=num_devices, dynamic_dma_scratch_size=2048)
    xT = nc.dram_tensor("xT", [P, (D // P - NF8) // 2, 2 * T], bf16,
                        kind="ExternalInput")
    Wp = nc.dram_tensor("Wp", [P, NDC, O], bf16, kind="ExternalInput")
    x8d = nc.dram_tensor("x8", [P, NT * 2 * NF8 * 64], fp8,
                         kind="ExternalInput")
    W8d = nc.dram_tensor("W8", [P, NOS * 4 * NF8 * P], fp8,
                         kind="ExternalInput")
    brep = nc.dram_tensor("brep", [P, O], f32, kind="ExternalInput")
    y = nc.dram_tensor("y", [T, O], f32, kind="ExternalOutput")

    with tile.TileContext(nc) as tc:
        with tc.tile_pool(name="xp", bufs=NDC // 2) as xp, \
             tc.tile_pool(name="w0p", bufs=NDC // 2) as w0p, \
             tc.tile_pool(name="wsp", bufs=2) as wsp, \
             tc.tile_pool(name="f8p", bufs=1) as f8p, \
             tc.tile_pool(name="bp", bufs=1) as bp, \
             tc.tile_pool(name="dp", bufs=1) as dp, \
             tc.tile_pool(name="s8p", bufs=2) as s8p, \
             tc.tile_pool(name="s8hp", bufs=2) as s8hp, \
             tc.tile_pool(name="yop", bufs=4) as yop, \
             tc.tile_pool(name="psp", bufs=7, space="PSUM") as psp, \
             tc.tile_pool(name="ps8p", bufs=1, space="PSUM") as ps8p:
            # warm-up spin: p-state ramp completes while first DMAs fly
            dummy = dp.tile([P, P], bf16)
            nc.gpsimd.memset(dummy[:], 0.0)
            wps = psp.tile([P, P], f32, tag="ps", name="warm")
            for _ in range(29):
                nc.tensor.matmul(wps[:], dummy[:], dummy[:],
                                 start=True, stop=True)

            # x d-tiles and o-slab-0 W chunks are fetched in dt-PAIRS
            # (one DMA per two tiles, side by side in the free dim) to
            # halve the serialized per-DMA HWDGE overhead; x is host-packed
            # [P, pair, 2T] so the DMA slices it like Wp (the AP pattern
            # proven correct on HW)
            xtp = [xp.tile([P, 2 * T], bf16, tag="x", name=f"x{i}")
                   for i in range(NDC // 2)]
            w0p_ = [w0p.tile([P, 2 * OS], bf16, tag="w0", name=f"w0_{i}")
                    for i in range(NDC // 2)]

            def xt_sl(dt, tt):
                return xtp[dt // 2][:, (dt % 2) * T + tt * P:
                                    (dt % 2) * T + (tt + 1) * P]

            def w0_sl(dt):
                return w0p_[dt // 2][:, (dt % 2) * OS:(dt % 2 + 1) * OS]

            for i in range(NDC // 2):
                nc.sync.dma_start(xtp[i][:], xT.ap()[:, i:i + 1, :])
                nc.sync.dma_start(w0p_[i][:],
                                  Wp.ap()[:, 2 * i:2 * i + 2, 0:OS])
            # fp8 operands: x8 whole + W8's o-slab-0 slice before ws1 so
            # o-slab 0's DoubleRow matmuls aren't gated on the big slabs
            x8t = f8p.tile([P, NT * 2 * NF8 * 64], fp8, tag="x8")
            W8t = f8p.tile([P, NOS * 4 * NF8 * P], fp8, tag="W8")
            W8OS = 4 * NF8 * P             # 1024 cols per o-slab
            nc.sync.dma_start(x8t[:], x8d.ap())
            nc.sync.dma_start(W8t[:, 0:W8OS], W8d.ap()[:, 0:W8OS])

            brt = bp.tile([P, O], f32)

            def load_bias(os_):
                nc.sync.dma_start(brt[:, os_ * OS:(os_ + 1) * OS],
                                  brep.ap()[:, os_ * OS:(os_ + 1) * OS])

            load_bias(0)
            ws = {}
            for os_ in (1, 2):
                ws[os_] = wsp.tile([P, NDC * OS], bf16, tag="ws",
                                   name=f"ws{os_}")
                nc.sync.dma_start(
                    ws[os_][:], Wp.ap()[:, :, os_ * OS:(os_ + 1) * OS])
                if os_ == 1:
                    nc.sync.dma_start(W8t[:, W8OS:], W8d.ap()[:, W8OS:])
                load_bias(os_)

            NPAIR = NF8 // 2

            def dr_mm(out_ap, tt, th, os_, oq, pair, start, stop):
                c0 = ((tt * 2 + th) * NPAIR + pair) * P
                lhsT = x8t[:, c0:c0 + P].rearrange(
                    "p (two m) -> p two m", two=2)
                r0 = ((os_ * 4 + oq) * NPAIR + pair) * 256
                rhs = W8t[:, r0:r0 + 256].rearrange(
                    "p (two f) -> p two f", two=2)
                nc.tensor.matmul(out_ap, lhsT, rhs, start=start, stop=stop,
                                 perf_mode=DR, skip_group_check=True)

            def th1_partial(tt, os_):
                """token-half-1 fp8 partial: DR (base 0) -> ACT -> SBUF
                partition-shift DMA to 64..127.  Returns s8h tile."""
                ps8 = ps8p.tile([64, OS], f32, tag="ps8",
                                name=f"ps8_{os_}_{tt}")
                for oq in range(4):
                    for pair in range(NPAIR):
                        dr_mm(ps8[:, oq * P:(oq + 1) * P], tt, 1, os_, oq,
                              pair, start=(pair == 0),
                              stop=(pair == NPAIR - 1))
                s8 = s8p.tile([64, OS], f32, tag="s8")
                nc.scalar.copy(s8[:], ps8[:])
                s8h = s8hp.tile([P, OS], f32, tag="s8h")
                nc.sync.dma_start(s8h[64:P, :], s8[:])
                return s8h

            def th0_into(py_t, tt, os_, off=0, cw=OS):
                oqs = list(range(off // P, (off + cw) // P))
                for i, oq in enumerate(oqs):
                    for pair in range(NPAIR):
                        dr_mm(py_t[0:64, oq * P - off:(oq + 1) * P - off],
                              tt, 0, os_, oq, pair, start=False,
                              stop=(i == len(oqs) - 1 and
                                    pair == NPAIR - 1))

            def evict(py_t, tt, os_, s8h):
                yo = yop.tile([P, OS], f32, tag="yo")
                nc.vector.tensor_add(yo[:], py_t[:],
                                     brt[:, os_ * OS:(os_ + 1) * OS])
                nc.vector.tensor_add(yo[64:P, :], yo[64:P, :], s8h[64:P, :])
                nc.sync.dma_start(
                    y.ap()[tt * P:(tt + 1) * P, os_ * OS:(os_ + 1) * OS],
                    yo[:])

            def bf16_chain(py_t, tt, w_ap, off=0, cw=OS):
                for dt in range(NDC):
                    nc.tensor.matmul(py_t[:], xt_sl(dt, tt),
                                     w_ap[:, dt * OS + off:dt * OS + off + cw],
                                     start=(dt == 0), stop=False)

            # o-slab 0: d-major over 7 t-tiles (psum pool is 7), so each
            # DMA round (x d-tile + W chunk) feeds 7 matmuls and the PE
            # never starves during the streaming phase
            py = [psp.tile([P, OS], f32, tag="ps", name=f"ps0_{tt}")
                  for tt in range(7)]
            for dt in range(NDC):
                for tt in range(7):
                    nc.tensor.matmul(py[tt][:],
                                     xt_sl(dt, tt),
                                     w0_sl(dt),
                                     start=(dt == 0), stop=False)

            def eb(tt):
                """finish + evict an o-slab-0 d-major tile"""
                s8h = th1_partial(tt, 0)
                th0_into(py[tt], tt, 0)
                evict(py[tt], tt, 0, s8h)

            def job(tt, os_, th1_late=False):
                """one full (tt, os) tile; th1 partial normally first (its
                ACT + shift DMA complete during the bf16 chain); th1_late
                puts it after the chain to space consecutive th1s apart in
                the o-slab-0 eviction zone (single ps8 buffer)"""
                s8h = None if th1_late else th1_partial(tt, os_)
                py_t = psp.tile([P, OS], f32, tag="ps",
                                name=f"ps{os_}_{tt}")
                if os_ == 0:
                    for dt in range(NDC):
                        nc.tensor.matmul(py_t[:],
                                         xt_sl(dt, tt),
                                         w0_sl(dt),
                                         start=(dt == 0), stop=False)
                else:
                    bf16_chain(py_t, tt, ws[os_])
                if th1_late:
                    s8h = th1_partial(tt, os_)
                th0_into(py_t, tt, os_)
                evict(py_t, tt, os_, s8h)

            def tail_job(tt, os_):
                s8h = th1_partial(tt, os_)
                # pre-combine bias + fp8 partial during the chains so each
                # chunk eviction is a single DVE add on the critical tail
                xc = s8hp.tile([P, OS], f32, tag="s8h", name="xcomb")
                b0 = os_ * OS
                nc.vector.tensor_copy(xc[0:64, :], brt[0:64, b0:b0 + OS])
                nc.vector.tensor_add(xc[64:P, :], brt[64:P, b0:b0 + OS],
                                     s8h[64:P, :])
                off = 0
                for h, cw in enumerate((256, 128, 128)):
                    ph = psp.tile([P, cw], f32, tag="ps",
                                  name=f"ps_tail{h}")
                    bf16_chain(ph, tt, ws[os_], off, cw)
                    th0_into(ph, tt, os_, off, cw)
                    yo = yop.tile([P, cw], f32, tag="yo2")
                    nc.vector.tensor_add(yo[:], ph[:], xc[:, off:off + cw])
                    nc.sync.dma_start(
                        y.ap()[tt * P:(tt + 1) * P, b0 + off:b0 + off + cw],
                        yo[:])
                    off += cw

            # interleave o-slab-0 evictions with the following full chains:
            # each eviction block is followed by a ~6.4us chain, which hides
            # the single-buffered fp8-scratch (ps8/s8/s8h) turnaround
            eb(0)
            job(7, 0)
            stagger = [(0, 1), (1, 1), (2, 1), (3, 1), (4, 1), (5, 1)]
            for k, (jt, jo) in enumerate(stagger):
                eb(k + 1)
                job(jt, jo)
            rest = [(6, 1), (7, 1)] + [(tt, os_) for os_ in range(2, NOS)
                                       for tt in range(NT)]
            for (jt, jo) in rest:
                if jo == NOS - 1 and jt == NT - 1:
                    tail_job(jt, jo)
                else:
                    job(jt, jo)
                if jt == NT - 1 and jo + 2 < NOS:
                    ws[jo + 2] = wsp.tile([P, NDC * OS], bf16, tag="ws",
                                          name=f"ws{jo + 2}")
                    nc.sync.dma_start(
                        ws[jo + 2][:],
                        Wp.ap()[:, :, (jo + 2) * OS:(jo + 3) * OS])
                    load_bias(jo + 2)
    nc.compile()
    return nc


_CACHED_NC = None


def _get_nc():
    global _CACHED_NC
    if _CACHED_NC is None:
        _CACHED_NC = build_kernel()
    return _CACHED_NC


def _fwht_rows(a, block):
    shape = a.shape
    a = a.reshape(-1, block).copy()
    h = 1
    while h < block:
        a = a.reshape(-1, block // (2 * h), 2, h)
        s = a[:, :, 0, :] + a[:, :, 1, :]
        d = a[:, :, 0, :] - a[:, :, 1, :]
        a = np.stack([s, d], axis=2)
        h *= 2
    return a.reshape(shape)


def kernel(x, W, b):
    x = np.asarray(x, dtype=np.float32)
    W = np.asarray(W, dtype=np.float32)
    b = np.asarray(b, dtype=np.float32)
    assert x.shape == (B, S, D) and W.shape == (O, D) and b.shape == (O,)

    nc = _get_nc()
    NDC = D // P - NF8
    e4m3 = ml_dtypes.float8_e4m3
    DS = NF8 * P                           # fp8 d-range (256)

    # W' = FWHT_1024(W rows)/32, then x1024 (exact host rescale at the end)
    Wf = _fwht_rows(W, HAD_BLOCK) * np.float32(SW / 32.0)
    WT = Wf.T                              # [d, o]
    Wpk = np.ascontiguousarray(
        WT[DS:].astype(ml_dtypes.bfloat16)
        .reshape(NDC, P, O).transpose(1, 0, 2))
    # fp8 blocks: [k][os][oq][pair][half][128]
    NPAIR = NF8 // 2
    W8v = WT[:DS].astype(e4m3).reshape(NPAIR, 2, P, O // OS, 4, P)
    W8pk = np.ascontiguousarray(
        W8v.transpose(2, 3, 4, 0, 1, 5)).reshape(P, (O // OS) * 4 * NF8 * P)
    brep = np.ascontiguousarray(
        np.broadcast_to((b * np.float32(SW)).reshape(1, O), (P, O)),
        dtype=np.float32)

    xf = x.reshape(B * S, D)
    in_maps = []
    for c in range(N_CORES):
        xc = xf[c * T_PER_CORE:(c + 1) * T_PER_CORE]
        xTc = xc[:, DS:].astype(ml_dtypes.bfloat16).T
        xTc = np.ascontiguousarray(
            xTc.reshape(NDC // 2, 2, P, T_PER_CORE).transpose(2, 0, 1, 3)
            .reshape(P, NDC // 2, 2 * T_PER_CORE))
        # fp8 part: [k][tt][th][pair][half][64]
        x8v = (xc[:, :DS].astype(e4m3).T
               .reshape(NPAIR, 2, P, T_PER_CORE // P, 2, 64))
        x8pk = np.ascontiguousarray(
            x8v.transpose(2, 3, 4, 0, 1, 5)).reshape(
                P, (T_PER_CORE // P) * 2 * NF8 * 64)
        in_maps.append({
            "xT": xTc, "Wp": Wpk, "x8": x8pk, "W8": W8pk, "brep": brep,
        })
    res = run_bass_kernel_spmd(nc, in_maps, core_ids=list(range(N_CORES)))
    yv = np.concatenate([res.results[c]["y"] for c in range(N_CORES)], axis=0)
    return (yv.reshape(B, S, O) * np.float32(1.0 / SW)).astype(
        np.float32, copy=False)


# revision 18
# speedup vs baseline: 1.0044x; 1.0004x over previous
"""NoisyHadamardLinear TRN2 kernel — bf16 main + fp8 DoubleRow hybrid.

Hadamard folded into W on host (y = x @ (W H_bd)^T + b), operands shipped
pre-transposed/pre-packed, one streaming matmul per core (data-parallel over
8192 tokens), plus: the first 4 of 32 contraction
d-blocks (d 0..511) are computed in fp8e4 with MatmulPerfMode.DoubleRow
(2 fp8 rows per PE cycle -> 0.5 cyc/row).  Scale bookkeeping: ALL W (bf16
and fp8) and the bias are pre-scaled x1024 on the host (free for bf16; gives
fp8 its mantissa window), x is unscaled, the host divides y by 1024 (exact).

DoubleRow mechanics: lhsT free [2, 64] = [A | B], rhs free [2, f] = [Wa|Wb],
out[64, f] = A^T Wa + B^T Wb (contraction depth 256 per instruction), and
the ISA only allows DR destinations on psum partitions 0..63.  So per
128-token tile: token-half 0 accumulates directly into the main psum bank
(partitions 0..63); token-half 1 goes to a scratch [64, x] psum at base 0,
is evicted by ACT to SBUF, partition-shifted 0..63 -> 64..127 by an
SBUF->SBUF DMA, and merged by a second DVE add at eviction time — all off
the PE critical path.
"""
import numpy as np
import ml_dtypes

import concourse.bacc as bacc
import concourse.mybir as mybir
import concourse.tile as tile
from concourse.bass_utils import run_bass_kernel_spmd

P = 128
OS = 512
bf16 = mybir.dt.bfloat16
fp8 = mybir.dt.float8e4
f32 = mybir.dt.float32

N_CORES = 8
B, S, D, O = 2, 4096, 4096, 4096
T_PER_CORE = (B * S) // N_CORES
HAD_BLOCK = 1024
NF8 = 4                    # fp8 d-blocks (d 0..511), must be even (DR pairs)
SW = 1024.0                # host-side W/bias scale (power of 2, exact)


def build_kernel(T=T_PER_CORE, D=D, O=O, num_devices=N_CORES):
    NDC = D // P - NF8                     # 30 clean (bf16) d-tiles
    NT = T // P                            # 8 t-tiles
    NOS = O // OS                          # 8 o-slabs
    DR = mybir.MatmulPerfMode.DoubleRow

    nc = bacc.Bacc("TRN2", target_bir_lowering=False, debug=False,
                   num_devices=num_devices, dynamic_dma_scratch_size=2048)
    xT = nc.dram_tensor("xT", [P, (D // P - NF8) // 2, 2 * T], bf16,
                        kind="ExternalInput")
    Wp = nc.dram_tensor("Wp", [P, NDC, O], bf16, kind="ExternalInput")
    x8d = nc.dram_tensor("x8", [P, NT * 2 * NF8 * 64], fp8,
                         kind="ExternalInput")
    W8d = nc.dram_tensor("W8", [P, NOS * 4 * NF8 * P], fp8,
                         kind="ExternalInput")
    brep = nc.dram_tensor("brep", [P, O], f32, kind="ExternalInput")
    y = nc.dram_tensor("y", [T, O], f32, kind="ExternalOutput")

    with tile.TileContext(nc) as tc:
        with tc.tile_pool(name="xp", bufs=NDC // 2) as xp, \
             tc.tile_pool(name="w0p", bufs=NDC // 2) as w0p, \
             tc.tile_pool(name="wsp", bufs=2) as wsp, \
             tc.tile_pool(name="f8p", bufs=1) as f8p, \
             tc.tile_pool(name="bp", bufs=1) as bp, \
             tc.tile_pool(name="dp", bufs=1) as dp, \
             tc.tile_pool(name="s8p", bufs=2) as s8p, \
             tc.tile_pool(name="s8hp", bufs=2) as s8hp, \
             tc.tile_pool(name="yop", bufs=4) as yop, \
             tc.tile_pool(name="psp", bufs=6, space="PSUM") as psp, \
             tc.tile_pool(name="ps8p", bufs=2, space="PSUM") as ps8p:
            # warm-up spin: p-state ramp completes while first DMAs fly
            dummy = dp.tile([P, P], bf16)
            nc.gpsimd.memset(dummy[:], 0.0)
            wps = psp.tile([P, P], f32, tag="ps", name="warm")
            for _ in range(29):
                nc.tensor.matmul(wps[:], dummy[:], dummy[:],
                                 start=True, stop=True)

            # x d-tiles and o-slab-0 W chunks are fetched in dt-PAIRS
            # (one DMA per two tiles, side by side in the free dim) to
            # halve the serialized per-DMA HWDGE overhead; x is host-packed
            # [P, pair, 2T] so the DMA slices it like Wp (the AP pattern
            # proven correct on HW)
            xtp = [xp.tile([P, 2 * T], bf16, tag="x", name=f"x{i}")
                   for i in range(NDC // 2)]
            w0p_ = [w0p.tile([P, 2 * OS], bf16, tag="w0", name=f"w0_{i}")
                    for i in range(NDC // 2)]

            def xt_sl(dt, tt):
                return xtp[dt // 2][:, (dt % 2) * T + tt * P:
                                    (dt % 2) * T + (tt + 1) * P]

            def w0_sl(dt):
                return w0p_[dt // 2][:, (dt % 2) * OS:(dt % 2 + 1) * OS]

            for i in range(NDC // 2):
                nc.sync.dma_start(xtp[i][:], xT.ap()[:, i:i + 1, :])
                nc.sync.dma_start(w0p_[i][:],
                                  Wp.ap()[:, 2 * i:2 * i + 2, 0:OS])
            # fp8 operands: x8 whole + W8's o-slab-0 slice before ws1 so
            # o-slab 0's DoubleRow matmuls aren't gated on the big slabs
            x8t = f8p.tile([P, NT * 2 * NF8 * 64], fp8, tag="x8")
            W8t = f8p.tile([P, NOS * 4 * NF8 * P], fp8, tag="W8")
            W8OS = 4 * NF8 * P             # 1024 cols per o-slab
            nc.sync.dma_start(x8t[:], x8d.ap())
            nc.sync.dma_start(W8t[:, 0:W8OS], W8d.ap()[:, 0:W8OS])

            brt = bp.tile([P, O], f32)

            def load_bias(os_):
                nc.sync.dma_start(brt[:, os_ * OS:(os_ + 1) * OS],
                                  brep.ap()[:, os_ * OS:(os_ + 1) * OS])

            load_bias(0)
            ws = {}
            for os_ in (1, 2):
                ws[os_] = wsp.tile([P, NDC * OS], bf16, tag="ws",
                                   name=f"ws{os_}")
                nc.sync.dma_start(
                    ws[os_][:], Wp.ap()[:, :, os_ * OS:(os_ + 1) * OS])
                if os_ == 1:
                    nc.sync.dma_start(W8t[:, W8OS:], W8d.ap()[:, W8OS:])
                load_bias(os_)

            NPAIR = NF8 // 2

            def dr_mm(out_ap, tt, th, os_, oq, pair, start, stop):
                c0 = ((tt * 2 + th) * NPAIR + pair) * P
                lhsT = x8t[:, c0:c0 + P].rearrange(
                    "p (two m) -> p two m", two=2)
                r0 = ((os_ * 4 + oq) * NPAIR + pair) * 256
                rhs = W8t[:, r0:r0 + 256].rearrange(
                    "p (two f) -> p two f", two=2)
                nc.tensor.matmul(out_ap, lhsT, rhs, start=start, stop=stop,
                                 perf_mode=DR, skip_group_check=True)

            def th1_partial(tt, os_):
                """token-half-1 fp8 partial: DR (base 0) -> ACT -> SBUF
                partition-shift DMA to 64..127.  Returns s8h tile."""
                ps8 = ps8p.tile([64, OS], f32, tag="ps8",
                                name=f"ps8_{os_}_{tt}")
                for oq in range(4):
                    for pair in range(NPAIR):
                        dr_mm(ps8[:, oq * P:(oq + 1) * P], tt, 1, os_, oq,
                              pair, start=(pair == 0),
                              stop=(pair == NPAIR - 1))
                s8 = s8p.tile([64, OS], f32, tag="s8")
                nc.scalar.copy(s8[:], ps8[:])
                s8h = s8hp.tile([P, OS], f32, tag="s8h")
                nc.sync.dma_start(s8h[64:P, :], s8[:])
                return s8h

            def th0_into(py_t, tt, os_, off=0, cw=OS):
                oqs = list(range(off // P, (off + cw) // P))
                for i, oq in enumerate(oqs):
                    for pair in range(NPAIR):
                        dr_mm(py_t[0:64, oq * P - off:(oq + 1) * P - off],
                              tt, 0, os_, oq, pair, start=False,
                              stop=(i == len(oqs) - 1 and
                                    pair == NPAIR - 1))

            def evict(py_t, tt, os_, s8h):
                yo = yop.tile([P, OS], f32, tag="yo")
                nc.vector.tensor_add(yo[:], py_t[:],
                                     brt[:, os_ * OS:(os_ + 1) * OS])
                nc.vector.tensor_add(yo[64:P, :], yo[64:P, :], s8h[64:P, :])
                nc.sync.dma_start(
                    y.ap()[tt * P:(tt + 1) * P, os_ * OS:(os_ + 1) * OS],
                    yo[:])

            def bf16_chain(py_t, tt, w_ap, off=0, cw=OS):
                for dt in range(NDC):
                    nc.tensor.matmul(py_t[:], xt_sl(dt, tt),
                                     w_ap[:, dt * OS + off:dt * OS + off + cw],
                                     start=(dt == 0), stop=False)

            # o-slab 0: d-major over 6 t-tiles (psum pool is 6; the freed
            # bank double-buffers the fp8 scratch).  With pair-loaded x/W
            # the DMA round is ~1.1us vs 6x213ns = 1.28us of PE work, so
            # the PE stays fed during the streaming phase
            py = [psp.tile([P, OS], f32, tag="ps", name=f"ps0_{tt}")
                  for tt in range(6)]
            for dt in range(NDC):
                for tt in range(6):
                    nc.tensor.matmul(py[tt][:],
                                     xt_sl(dt, tt),
                                     w0_sl(dt),
                                     start=(dt == 0), stop=False)

            def eb(tt):
                """finish + evict an o-slab-0 d-major tile"""
                s8h = th1_partial(tt, 0)
                th0_into(py[tt], tt, 0)
                evict(py[tt], tt, 0, s8h)

            def job(tt, os_, th1_late=False):
                """one full (tt, os) tile; th1 partial normally first (its
                ACT + shift DMA complete during the bf16 chain); th1_late
                puts it after the chain to space consecutive th1s apart in
                the o-slab-0 eviction zone (single ps8 buffer)"""
                s8h = None if th1_late else th1_partial(tt, os_)
                py_t = psp.tile([P, OS], f32, tag="ps",
                                name=f"ps{os_}_{tt}")
                if os_ == 0:
                    for dt in range(NDC):
                        nc.tensor.matmul(py_t[:],
                                         xt_sl(dt, tt),
                                         w0_sl(dt),
                                         start=(dt == 0), stop=False)
                else:
                    bf16_chain(py_t, tt, ws[os_])
                if th1_late:
                    s8h = th1_partial(tt, os_)
                th0_into(py_t, tt, os_)
                evict(py_t, tt, os_, s8h)

            def tail_job(tt, os_):
                s8h = th1_partial(tt, os_)
                # pre-combine bias + fp8 partial during the chains so each
                # chunk eviction is a single DVE add on the critical tail
                xc = s8hp.tile([P, OS], f32, tag="s8h", name="xcomb")
                b0 = os_ * OS
                nc.vector.tensor_copy(xc[0:64, :], brt[0:64, b0:b0 + OS])
                nc.vector.tensor_add(xc[64:P, :], brt[64:P, b0:b0 + OS],
                                     s8h[64:P, :])
                off = 0
                for h, cw in enumerate((256, 128, 128)):
                    ph = psp.tile([P, cw], f32, tag="ps",
                                  name=f"ps_tail{h}")
                    bf16_chain(ph, tt, ws[os_], off, cw)
                    th0_into(ph, tt, os_, off, cw)
                    yo = yop.tile([P, cw], f32, tag="yo2")
                    nc.vector.tensor_add(yo[:], ph[:], xc[:, off:off + cw])
                    nc.sync.dma_start(
                        y.ap()[tt * P:(tt + 1) * P, b0 + off:b0 + off + cw],
                        yo[:])
                    off += cw

            # interleave o-slab-0 evictions with the following full chains:
            # each eviction block is followed by a ~6.4us chain, which hides
            # the single-buffered fp8-scratch (ps8/s8/s8h) turnaround
            eb(0)
            job(6, 0)
            stagger = [(7, 0), (0, 1), (1, 1), (2, 1), (3, 1)]
            for k, (jt, jo) in enumerate(stagger):
                eb(k + 1)
                job(jt, jo)
            rest = [(4, 1), (5, 1), (6, 1), (7, 1)] + \
                   [(tt, os_) for os_ in range(2, NOS) for tt in range(NT)]
            for (jt, jo) in rest:
                if jo == NOS - 1 and jt == NT - 1:
                    tail_job(jt, jo)
                else:
                    job(jt, jo)
                if jt == NT - 1 and jo + 2 < NOS:
                    ws[jo + 2] = wsp.tile([P, NDC * OS], bf16, tag="ws",
                                          name=f"ws{jo + 2}")
                    nc.sync.dma_start(
                        ws[jo + 2][:],
                        Wp.ap()[:, :, (jo + 2) * OS:(jo + 3) * OS])
                    load_bias(jo + 2)
    nc.compile()
    return nc


_CACHED_NC = None


def _get_nc():
    global _CACHED_NC
    if _CACHED_NC is None:
        _CACHED_NC = build_kernel()
    return _CACHED_NC


def _fwht_rows(a, block):
    shape = a.shape
    a = a.reshape(-1, block).copy()
    h = 1
    while h < block:
        a = a.reshape(-1, block // (2 * h), 2, h)
        s = a[:, :, 0, :] + a[:, :, 1, :]
        d = a[:, :, 0, :] - a[:, :, 1, :]
        a = np.stack([s, d], axis=2)
        h *= 2
    return a.reshape(shape)


def kernel(x, W, b):
    x = np.asarray(x, dtype=np.float32)
    W = np.asarray(W, dtype=np.float32)
    b = np.asarray(b, dtype=np.float32)
    assert x.shape == (B, S, D) and W.shape == (O, D) and b.shape == (O,)

    nc = _get_nc()
    NDC = D // P - NF8
    e4m3 = ml_dtypes.float8_e4m3
    DS = NF8 * P                           # fp8 d-range (256)

    # W' = FWHT_1024(W rows)/32, then x1024 (exact host rescale at the end)
    Wf = _fwht_rows(W, HAD_BLOCK) * np.float32(SW / 32.0)
    WT = Wf.T                              # [d, o]
    Wpk = np.ascontiguousarray(
        WT[DS:].astype(ml_dtypes.bfloat16)
        .reshape(NDC, P, O).transpose(1, 0, 2))
    # fp8 blocks: [k][os][oq][pair][half][128]
    NPAIR = NF8 // 2
    W8v = WT[:DS].astype(e4m3).reshape(NPAIR, 2, P, O // OS, 4, P)
    W8pk = np.ascontiguousarray(
        W8v.transpose(2, 3, 4, 0, 1, 5)).reshape(P, (O // OS) * 4 * NF8 * P)
    brep = np.ascontiguousarray(
        np.broadcast_to((b * np.float32(SW)).reshape(1, O), (P, O)),
        dtype=np.float32)

    xf = x.reshape(B * S, D)
    in_maps = []
    for c in range(N_CORES):
        xc = xf[c * T_PER_CORE:(c + 1) * T_PER_CORE]
        xTc = xc[:, DS:].astype(ml_dtypes.bfloat16).T
        xTc = np.ascontiguousarray(
            xTc.reshape(NDC // 2, 2, P, T_PER_CORE).transpose(2, 0, 1, 3)
            .reshape(P, NDC // 2, 2 * T_PER_CORE))
        # fp8 part: [k][tt][th][pair][half][64]
        x8v = (xc[:, :DS].astype(e4m3).T
               .reshape(NPAIR, 2, P, T_PER_CORE // P, 2, 64))
        x8pk = np.ascontiguousarray(
            x8v.transpose(2, 3, 4, 0, 1, 5)).reshape(
                P, (T_PER_CORE // P) * 2 * NF8 * 64)
        in_maps.append({
            "xT": xTc, "Wp": Wpk, "x8": x8pk, "W8": W8pk, "brep": brep,
        })
    res = run_bass_kernel_spmd(nc, in_maps, core_ids=list(range(N_CORES)))
    yv = np.concatenate([res.results[c]["y"] for c in range(N_CORES)], axis=0)
    return (yv.reshape(B, S, O) * np.float32(1.0 / SW)).astype(
        np.float32, copy=False)
